# revision 17
# baseline (speedup 1.0000x reference)
"""CrossAttention kernel for 8 Trainium2 NeuronCores.

Data-parallel over batch: B=16 batches -> 2 per core. Each core computes the
full cross-attention for its 2 batches; outputs are concatenated on host.

Per-core dataflow (all matmuls in float32r, full-rate on the PE):
  x      [2,4096,512] --PE transpose--> x^T tiles [512, nq]
  Q^T    = Wq^T @ x^T            (Wq pre-scaled by 1/8 on host)
  ctx^T  --PE transpose--> K^T = Wk^T @ ctx^T,  V = ctx @ Wv
  S^T    = (K_h^T).T @ Q_h^T     [77, nq]  per head
  expS   = exp(S^T)              (no max-subtract; |S| <~ 6 so fp32 exp safe)
  den    = ones^T @ expS         [1, nq] packed 4 heads/psum-bank at rows {0,32,64,96}
  O^T    = V_h.T @ expS          [64, nq], head pairs packed into [128, nq]
  norm   : O^T * (1/den) with 1/den broadcast across partitions via GPSIMD
  out    = (O^T).T @ Wo + bo     [nq, 512]
"""

import os
import sys

for _p in ("/opt/trn_rl_repo",):
    if _p not in sys.path:
        sys.path.insert(0, _p)

import numpy as np

import concourse.bass as bass
import concourse.bacc as bacc
import concourse.mybir as mybir
import concourse.tile as tile
from concourse.bass_utils import run_bass_kernel_spmd

# Problem constants (hardcoded per contract)
B, NQ, NK = 16, 4096, 77
DQ, DC = 512, 768
H, DH = 8, 64
INNER = H * DH  # 512
SCALE = DH ** -0.5  # 1/8
NCORES = 8
BLOC = B // NCORES  # 2 batches per core

F32 = mybir.dt.float32
F16 = mybir.dt.float16
MMDT = mybir.dt.float32r  # full-rate fp32 matmul dtype

TQ = 512          # nq tile (free dim of most matmuls)
NTILES = NQ // TQ  # 8 per batch


def _build_nc():
    nc = bacc.Bacc("TRN2", target_bir_lowering=False, debug=False)

    xT_l = nc.dram_tensor("xT_l", [BLOC, DQ, NQ], F16, kind="ExternalInput")
    ctx_l = nc.dram_tensor("ctx_l", [BLOC, NK, DC], F32, kind="ExternalInput")
    wq = nc.dram_tensor("wq", [DQ, INNER], F16, kind="ExternalInput")
    wk = nc.dram_tensor("wk", [DC, INNER], F16, kind="ExternalInput")
    wv = nc.dram_tensor("wv", [DC, INNER], F16, kind="ExternalInput")
    wo = nc.dram_tensor("wo", [INNER, DQ], F16, kind="ExternalInput")
    bo = nc.dram_tensor("bo", [DQ], F32, kind="ExternalInput")
    ident32 = nc.dram_tensor("ident32", [128, 128], F32, kind="ExternalInput")
    ones77 = nc.dram_tensor("ones77", [NK, 64], F16, kind="ExternalInput")
    out_l = nc.dram_tensor("out_l", [BLOC, NQ, DQ], F32, kind="ExternalOutput")

    KQ = DQ // 128   # 4 contraction chunks for Wq
    KC = DC // 128   # 6 contraction chunks for Wk/Wv
    CI = INNER // 128  # 4 inner chunks

    with tile.TileContext(nc) as tc:
        with (
            tc.tile_pool(name="consts", bufs=1) as consts,
            tc.tile_pool(name="kv", bufs=1) as kv,
            tc.tile_pool(name="xp", bufs=3) as xp,
            tc.tile_pool(name="ep", bufs=12) as ep,
            tc.tile_pool(name="dp", bufs=4) as dp,
            tc.tile_pool(name="rp", bufs=8) as rp,
            tc.tile_pool(name="op", bufs=2) as op,
            tc.tile_pool(name="fp", bufs=2) as fp,
            tc.tile_pool(name="ps", bufs=8, space="PSUM") as ps,
        ):
            # ---- constants / weights (ctx-critical loads first) ----
            ident32_sb = consts.tile([128, 128], F32)
            nc.scalar.dma_start(ident32_sb[:], ident32[:])
            ones77_sb = consts.tile([NK, 64], F16)
            nc.scalar.dma_start(ones77_sb[:], ones77[:])
            wq_sb = consts.tile([128, KQ, INNER], F16)
            nc.sync.dma_start(
                wq_sb[:], wq.ap().rearrange("(k p) n -> p k n", p=128))
            wk_sb = consts.tile([128, KC, INNER], F16)
            nc.sync.dma_start(
                wk_sb[:], wk.ap().rearrange("(k p) n -> p k n", p=128))
            wv_sb = consts.tile([128, KC, INNER], F16)
            nc.sync.dma_start(
                wv_sb[:], wv.ap().rearrange("(k p) n -> p k n", p=128))
            wo_sb = consts.tile([128, CI, DQ], F16)
            nc.sync.dma_start(
                wo_sb[:], wo.ap().rearrange("(k p) n -> p k n", p=128))
            # bias broadcast to all 128 partitions via step-0 DMA
            bo_bc = consts.tile([128, DQ], F32)
            nc.gpsimd.dma_start(
                out=bo_bc[:], in_=bo.ap()[None, :].broadcast_to([128, DQ]))

            # ---- context projections (K^T for both batches, V per batch) ----
            ctxT_sb = kv.tile([128, KC, BLOC * NK], F16)
            for b in range(BLOC):
                ctx_sb = kv.tile([NK, DC], F32, tag="ctx_sb")
                nc.scalar.dma_start(ctx_sb[:], ctx_l[b])
                for c in range(KC):
                    ct_ps = ps.tile([128, NK], F32, tag="ps")
                    nc.tensor.transpose(
                        ct_ps[:], ctx_sb[:, c * 128:(c + 1) * 128],
                        ident32_sb[:NK, :NK])
                    nc.vector.tensor_copy(
                        ctxT_sb[:, c, b * NK:(b + 1) * NK], ct_ps[:])

            kt_sb = kv.tile([128, CI, BLOC * NK], F16)
            for c in range(CI):
                kt_ps = ps.tile([128, BLOC * NK], F32, tag="ps")
                for k in range(KC):
                    nc.tensor.matmul(
                        kt_ps[:], wk_sb[:, k, c * 128:(c + 1) * 128],
                        ctxT_sb[:, k, :], start=(k == 0), stop=(k == KC - 1))
                nc.scalar.activation(
                    kt_sb[:, c, :], kt_ps[:],
                    mybir.ActivationFunctionType.Copy)

            v_sb = kv.tile([NK, BLOC, INNER], F16)
            for b in range(BLOC):
                v_ps = ps.tile([NK, INNER], F32, tag="ps")
                for k in range(KC):
                    nc.tensor.matmul(
                        v_ps[:], ctxT_sb[:, k, b * NK:(b + 1) * NK],
                        wv_sb[:, k, :], start=(k == 0), stop=(k == KC - 1))
                nc.scalar.activation(
                    v_sb[:, b, :], v_ps[:],
                    mybir.ActivationFunctionType.Copy)

            # ---- main loop over (batch, nq tile) ----
            for b in range(BLOC):
                for t in range(NTILES):
                    nq0 = t * TQ
                    # load x^T tile directly (x transposed on host)
                    xT_sb = xp.tile([128, KQ, TQ], F16, tag="xT")
                    xT_src = xT_l[b].rearrange("(c p) n -> p c n", p=128)[
                        :, :, nq0:nq0 + TQ]
                    if (b * NTILES + t) % 2 == 0:
                        nc.scalar.dma_start(xT_sb[:], xT_src)
                    else:
                        nc.gpsimd.dma_start(xT_sb[:], xT_src)
                    # Q^T = Wq^T @ x^T (Wq pre-scaled by 1/8 on host)
                    qt_sb = xp.tile([128, CI, TQ], F16, tag="qt")
                    for c in range(CI):
                        q_ps = ps.tile([128, TQ], F32, tag="ps")
                        for k in range(KQ):
                            nc.tensor.matmul(
                                q_ps[:], wq_sb[:, k, c * 128:(c + 1) * 128],
                                xT_sb[:, k, :],
                                start=(k == 0), stop=(k == KQ - 1))
                        nc.scalar.activation(
                            qt_sb[:, c, :], q_ps[:],
                            mybir.ActivationFunctionType.Copy)

                    # attention: S^T, exp
                    e_sbs = []
                    for h in range(H):
                        c, r = h // 2, (h % 2) * 64
                        s_ps = ps.tile([NK, TQ], F32, tag="ps")
                        nc.tensor.matmul(
                            s_ps[:],
                            kt_sb[r:r + DH, c, b * NK:(b + 1) * NK],
                            qt_sb[r:r + DH, c, :])
                        e_sb = ep.tile([NK, TQ], F16, tag="expS")
                        nc.scalar.activation(
                            e_sb[:], s_ps[:], mybir.ActivationFunctionType.Exp)
                        e_sbs.append(e_sb)

                    # O^T head-pairs packed [128, TQ], normalized by 1/den.
                    # den matmuls replicate each head's denominator across a
                    # 64-row strip so the pair bank matches the O pair layout.
                    ot_sb = op.tile([128, CI, TQ], F16, tag="ot")
                    for g in range(H // 2):
                        o2_ps = ps.tile([128, TQ], F32, tag="ps")
                        d_ps = ps.tile([128, TQ], F32, tag="ps")
                        for half in range(2):
                            h = 2 * g + half
                            nc.tensor.matmul(
                                o2_ps[half * 64:(half + 1) * 64, :],
                                v_sb[:, b, h * DH:(h + 1) * DH],
                                e_sbs[h][:])
                            nc.tensor.matmul(
                                d_ps[half * 64:(half + 1) * 64, :],
                                ones77_sb[:], e_sbs[h][:],
                                tile_position=(0, half * 64))
                        rdbc = rp.tile([128, TQ], F32, tag="rdbc")
                        nc.vector.reciprocal_approx_fast(rdbc[:], d_ps[:])
                        nc.vector.tensor_mul(ot_sb[:, g, :], o2_ps[:], rdbc[:])

                    # out = (O^T).T @ Wo + bo
                    f_sb = fp.tile([128, 4, DQ], F32, tag="fin")
                    for j in range(4):
                        f_ps = ps.tile([128, DQ], F32, tag="ps")
                        for c in range(CI):
                            nc.tensor.matmul(
                                f_ps[:], ot_sb[:, c, j * 128:(j + 1) * 128],
                                wo_sb[:, c, :],
                                start=(c == 0), stop=(c == CI - 1))
                        nc.vector.tensor_add(f_sb[:, j, :], f_ps[:], bo_bc[:])
                    st_eng = [nc.sync, nc.scalar, nc.gpsimd][
                        (b * NTILES + t) % 3]
                    st_eng.dma_start(
                        out_l[b, nq0:nq0 + TQ, :].rearrange(
                            "(j p) n -> p j n", p=128),
                        f_sb[:])

    nc.compile()
    return nc


_NC_CACHE = {}


def _get_nc():
    if "nc" not in _NC_CACHE:
        _NC_CACHE["nc"] = _build_nc()
    return _NC_CACHE["nc"]


def _make_in_maps(x, context, Wq, Wk, Wv, Wo, bo):
    f = np.float32
    shared = {
        "wq": (np.ascontiguousarray(Wq, dtype=f) * np.float32(SCALE)).astype(np.float16),
        "wk": np.ascontiguousarray(Wk, dtype=f).astype(np.float16),
        "wv": np.ascontiguousarray(Wv, dtype=f).astype(np.float16),
        "wo": np.ascontiguousarray(Wo, dtype=f).astype(np.float16),
        "bo": np.ascontiguousarray(bo, dtype=f),
        "ident32": np.eye(128, dtype=f),
        "ones77": np.ones((NK, 64), dtype=np.float16),
    }
    in_maps = []
    for i in range(NCORES):
        m = dict(shared)
        m["xT_l"] = np.ascontiguousarray(
            np.asarray(x[BLOC * i:BLOC * (i + 1)], dtype=f).transpose(
                0, 2, 1)).astype(np.float16)
        m["ctx_l"] = np.ascontiguousarray(
            context[BLOC * i:BLOC * (i + 1)], dtype=f)
        in_maps.append(m)
    return in_maps


def run(x, context, Wq, Wk, Wv, Wo, bo, trace=False, **trace_kwargs):
    nc = _get_nc()
    in_maps = _make_in_maps(x, context, Wq, Wk, Wv, Wo, bo)
    res = run_bass_kernel_spmd(
        nc, in_maps, list(range(NCORES)), trace=trace, **trace_kwargs)
    out = np.concatenate(
        [np.asarray(res.results[i]["out_l"]) for i in range(NCORES)], axis=0)
    return out.astype(np.float32), res


def kernel(x, context, Wq, Wk, Wv, Wo, bo):
    out, _ = run(x, context, Wq, Wk, Wv, Wo, bo, trace=False)
    return out


# revision 18
# speedup vs baseline: 1.0147x; 1.0147x over previous
"""CrossAttention kernel for 8 Trainium2 NeuronCores.

Data-parallel over batch: B=16 batches -> 2 per core. Each core computes the
full cross-attention for its 2 batches; outputs are concatenated on host.

Per-core dataflow (all matmuls in float32r, full-rate on the PE):
  x      [2,4096,512] --PE transpose--> x^T tiles [512, nq]
  Q^T    = Wq^T @ x^T            (Wq pre-scaled by 1/8 on host)
  ctx^T  --PE transpose--> K^T = Wk^T @ ctx^T,  V = ctx @ Wv
  S^T    = (K_h^T).T @ Q_h^T     [77, nq]  per head
  expS   = exp(S^T)              (no max-subtract; |S| <~ 6 so fp32 exp safe)
  den    = ones^T @ expS         [1, nq] packed 4 heads/psum-bank at rows {0,32,64,96}
  O^T    = V_h.T @ expS          [64, nq], head pairs packed into [128, nq]
  norm   : O^T * (1/den) with 1/den broadcast across partitions via GPSIMD
  out    = (O^T).T @ Wo + bo     [nq, 512]
"""

import os
import sys

for _p in ("/opt/trn_rl_repo",):
    if _p not in sys.path:
        sys.path.insert(0, _p)

import numpy as np

import concourse.bass as bass
import concourse.bacc as bacc
import concourse.mybir as mybir
import concourse.tile as tile
from concourse.bass_utils import run_bass_kernel_spmd

# Problem constants (hardcoded per contract)
B, NQ, NK = 16, 4096, 77
DQ, DC = 512, 768
H, DH = 8, 64
INNER = H * DH  # 512
SCALE = DH ** -0.5  # 1/8
NCORES = 8
BLOC = B // NCORES  # 2 batches per core

F32 = mybir.dt.float32
F16 = mybir.dt.float16
MMDT = mybir.dt.float32r  # full-rate fp32 matmul dtype

TQ = 512          # nq tile (free dim of most matmuls)
NTILES = NQ // TQ  # 8 per batch


def _build_nc():
    nc = bacc.Bacc("TRN2", target_bir_lowering=False, debug=False)

    xT_l = nc.dram_tensor("xT_l", [BLOC, DQ, NQ], F16, kind="ExternalInput")
    ctx_l = nc.dram_tensor("ctx_l", [BLOC, NK, DC], F32, kind="ExternalInput")
    wq = nc.dram_tensor("wq", [DQ, INNER], F16, kind="ExternalInput")
    wk = nc.dram_tensor("wk", [DC, INNER], F16, kind="ExternalInput")
    wv = nc.dram_tensor("wv", [DC, INNER], F16, kind="ExternalInput")
    wo = nc.dram_tensor("wo", [INNER, DQ], F16, kind="ExternalInput")
    bo = nc.dram_tensor("bo", [DQ], F32, kind="ExternalInput")
    ident32 = nc.dram_tensor("ident32", [128, 128], F32, kind="ExternalInput")
    ones77 = nc.dram_tensor("ones77", [NK, 64], F16, kind="ExternalInput")
    out_l = nc.dram_tensor("out_l", [BLOC, NQ, DQ], F32, kind="ExternalOutput")

    KQ = DQ // 128   # 4 contraction chunks for Wq
    KC = DC // 128   # 6 contraction chunks for Wk/Wv
    CI = INNER // 128  # 4 inner chunks

    with tile.TileContext(nc) as tc:
        with (
            tc.tile_pool(name="consts", bufs=1) as consts,
            tc.tile_pool(name="kv", bufs=1) as kv,
            tc.tile_pool(name="xp", bufs=3) as xp,
            tc.tile_pool(name="ep", bufs=12) as ep,
            tc.tile_pool(name="dp", bufs=4) as dp,
            tc.tile_pool(name="rp", bufs=8) as rp,
            tc.tile_pool(name="op", bufs=2) as op,
            tc.tile_pool(name="fp", bufs=2) as fp,
            tc.tile_pool(name="ps", bufs=8, space="PSUM") as ps,
        ):
            # ---- constants / weights (ctx-critical loads first) ----
            ident32_sb = consts.tile([128, 128], F32)
            nc.scalar.dma_start(ident32_sb[:], ident32[:])
            ones77_sb = consts.tile([NK, 64], F16)
            nc.scalar.dma_start(ones77_sb[:], ones77[:])
            ctx_sb = kv.tile([NK, BLOC, DC], F32)
            nc.sync.dma_start(
                ctx_sb[:], ctx_l.ap().rearrange("b k c -> k b c"))
            wq_sb = consts.tile([128, KQ, INNER], F16)
            nc.sync.dma_start(
                wq_sb[:], wq.ap().rearrange("(k p) n -> p k n", p=128))
            wk_sb = consts.tile([128, KC, INNER], F16)
            nc.sync.dma_start(
                wk_sb[:], wk.ap().rearrange("(k p) n -> p k n", p=128))
            wv_sb = consts.tile([128, KC, INNER], F16)
            nc.sync.dma_start(
                wv_sb[:], wv.ap().rearrange("(k p) n -> p k n", p=128))
            wo_sb = consts.tile([128, CI, DQ], F16)
            nc.sync.dma_start(
                wo_sb[:], wo.ap().rearrange("(k p) n -> p k n", p=128))
            # bias broadcast to all 128 partitions via step-0 DMA
            bo_bc = consts.tile([128, DQ], F32)
            nc.gpsimd.dma_start(
                out=bo_bc[:], in_=bo.ap()[None, :].broadcast_to([128, DQ]))

            # ---- context projections (K^T for both batches, V per batch) ----
            ctxT_sb = kv.tile([128, KC, BLOC * NK], F16)
            for b in range(BLOC):
                for c in range(KC):
                    ct_ps = ps.tile([128, NK], F32, tag="ps")
                    nc.tensor.transpose(
                        ct_ps[:], ctx_sb[:, b, c * 128:(c + 1) * 128],
                        ident32_sb[:NK, :NK])
                    nc.vector.tensor_copy(
                        ctxT_sb[:, c, b * NK:(b + 1) * NK], ct_ps[:])

            kt_sb = kv.tile([128, CI, BLOC * NK], F16)
            for c in range(CI):
                kt_ps = ps.tile([128, BLOC * NK], F32, tag="ps")
                for k in range(KC):
                    nc.tensor.matmul(
                        kt_ps[:], wk_sb[:, k, c * 128:(c + 1) * 128],
                        ctxT_sb[:, k, :], start=(k == 0), stop=(k == KC - 1))
                nc.scalar.activation(
                    kt_sb[:, c, :], kt_ps[:],
                    mybir.ActivationFunctionType.Copy)

            v_sb = kv.tile([NK, BLOC, INNER], F16)
            for b in range(BLOC):
                v_ps = ps.tile([NK, INNER], F32, tag="ps")
                for k in range(KC):
                    nc.tensor.matmul(
                        v_ps[:], ctxT_sb[:, k, b * NK:(b + 1) * NK],
                        wv_sb[:, k, :], start=(k == 0), stop=(k == KC - 1))
                nc.scalar.activation(
                    v_sb[:, b, :], v_ps[:],
                    mybir.ActivationFunctionType.Copy)

            # ---- main loop over (batch, nq tile) ----
            for b in range(BLOC):
                for t in range(NTILES):
                    nq0 = t * TQ
                    # load x^T tile directly (x transposed on host)
                    xT_sb = xp.tile([128, KQ, TQ], F16, tag="xT")
                    xT_src = xT_l[b].rearrange("(c p) n -> p c n", p=128)[
                        :, :, nq0:nq0 + TQ]
                    if (b * NTILES + t) % 2 == 0:
                        nc.scalar.dma_start(xT_sb[:], xT_src)
                    else:
                        nc.gpsimd.dma_start(xT_sb[:], xT_src)
                    # Q^T = Wq^T @ x^T (Wq pre-scaled by 1/8 on host)
                    qt_sb = xp.tile([128, CI, TQ], F16, tag="qt")
                    for c in range(CI):
                        q_ps = ps.tile([128, TQ], F32, tag="ps")
                        for k in range(KQ):
                            nc.tensor.matmul(
                                q_ps[:], wq_sb[:, k, c * 128:(c + 1) * 128],
                                xT_sb[:, k, :],
                                start=(k == 0), stop=(k == KQ - 1))
                        nc.scalar.activation(
                            qt_sb[:, c, :], q_ps[:],
                            mybir.ActivationFunctionType.Copy)

                    # attention: S^T, exp
                    e_sbs = []
                    for h in range(H):
                        c, r = h // 2, (h % 2) * 64
                        s_ps = ps.tile([NK, TQ], F32, tag="ps")
                        nc.tensor.matmul(
                            s_ps[:],
                            kt_sb[r:r + DH, c, b * NK:(b + 1) * NK],
                            qt_sb[r:r + DH, c, :])
                        e_sb = ep.tile([NK, TQ], F16, tag="expS")
                        nc.scalar.activation(
                            e_sb[:], s_ps[:], mybir.ActivationFunctionType.Exp)
                        e_sbs.append(e_sb)

                    # O^T head-pairs packed [128, TQ], normalized by 1/den.
                    # den matmuls replicate each head's denominator across a
                    # 64-row strip so the pair bank matches the O pair layout.
                    ot_sb = op.tile([128, CI, TQ], F16, tag="ot")
                    for g in range(H // 2):
                        o2_ps = ps.tile([128, TQ], F32, tag="ps")
                        d_ps = ps.tile([128, TQ], F32, tag="ps")
                        for half in range(2):
                            h = 2 * g + half
                            nc.tensor.matmul(
                                o2_ps[half * 64:(half + 1) * 64, :],
                                v_sb[:, b, h * DH:(h + 1) * DH],
                                e_sbs[h][:])
                            nc.tensor.matmul(
                                d_ps[half * 64:(half + 1) * 64, :],
                                ones77_sb[:], e_sbs[h][:],
                                tile_position=(0, half * 64))
                        rdbc = rp.tile([128, TQ], F32, tag="rdbc")
                        nc.vector.reciprocal_approx_fast(rdbc[:], d_ps[:])
                        nc.vector.tensor_mul(ot_sb[:, g, :], o2_ps[:], rdbc[:])

                    # out = (O^T).T @ Wo + bo
                    f_sb = fp.tile([128, 4, DQ], F32, tag="fin")
                    for j in range(4):
                        f_ps = ps.tile([128, DQ], F32, tag="ps")
                        for c in range(CI):
                            nc.tensor.matmul(
                                f_ps[:], ot_sb[:, c, j * 128:(j + 1) * 128],
                                wo_sb[:, c, :],
                                start=(c == 0), stop=(c == CI - 1))
                        nc.vector.tensor_add(f_sb[:, j, :], f_ps[:], bo_bc[:])
                    st_eng = [nc.sync, nc.scalar, nc.gpsimd][
                        (b * NTILES + t) % 3]
                    st_eng.dma_start(
                        out_l[b, nq0:nq0 + TQ, :].rearrange(
                            "(j p) n -> p j n", p=128),
                        f_sb[:])

    nc.compile()
    return nc


_NC_CACHE = {}


def _get_nc():
    if "nc" not in _NC_CACHE:
        _NC_CACHE["nc"] = _build_nc()
    return _NC_CACHE["nc"]


def _make_in_maps(x, context, Wq, Wk, Wv, Wo, bo):
    f = np.float32
    shared = {
        "wq": (np.ascontiguousarray(Wq, dtype=f) * np.float32(SCALE)).astype(np.float16),
        "wk": np.ascontiguousarray(Wk, dtype=f).astype(np.float16),
        "wv": np.ascontiguousarray(Wv, dtype=f).astype(np.float16),
        "wo": np.ascontiguousarray(Wo, dtype=f).astype(np.float16),
        "bo": np.ascontiguousarray(bo, dtype=f),
        "ident32": np.eye(128, dtype=f),
        "ones77": np.ones((NK, 64), dtype=np.float16),
    }
    in_maps = []
    for i in range(NCORES):
        m = dict(shared)
        m["xT_l"] = np.ascontiguousarray(
            np.asarray(x[BLOC * i:BLOC * (i + 1)], dtype=f).transpose(
                0, 2, 1)).astype(np.float16)
        m["ctx_l"] = np.ascontiguousarray(
            context[BLOC * i:BLOC * (i + 1)], dtype=f)
        in_maps.append(m)
    return in_maps


def run(x, context, Wq, Wk, Wv, Wo, bo, trace=False, **trace_kwargs):
    nc = _get_nc()
    in_maps = _make_in_maps(x, context, Wq, Wk, Wv, Wo, bo)
    res = run_bass_kernel_spmd(
        nc, in_maps, list(range(NCORES)), trace=trace, **trace_kwargs)
    out = np.concatenate(
        [np.asarray(res.results[i]["out_l"]) for i in range(NCORES)], axis=0)
    return out.astype(np.float32), res


def kernel(x, context, Wq, Wk, Wv, Wo, bo):
    out, _ = run(x, context, Wq, Wk, Wv, Wo, bo, trace=False)
    return out
